# revision 1
# baseline (speedup 1.0000x reference)
"""Trainium2 Bass kernel: e3nn-style CudaTensorProduct (CG tensor product).

out[b, o] = sum_nnz cb * in1[b, i1] * in2[b, i2]

in1: [8192, 288] = 32 channels each of l1=0,1,2 (dims 1/3/5)
in2: [8192, 9]   = spherical harmonics l2=0..2
out: [8192, 2592]

Device formulation (per core, batch slice of 1024, data parallel over 8 cores):
  For each group g (l1=0,1,2) the CG coefficients are channel-independent.
  Let p = (g, m1, j) index the 81 = 9*(1+3+5) "product rows" and
  q = (g, o3) index the 81 = 9+27+45 output rows per channel.
    Z[p, (c, b)]   = in1[b, col(g,c,m1)] * in2[b, j]
    out[q, (c, b)] = sum_p T_all[p, q] * Z[p, (c, b)]
  On device:
    in1rep = S.T @ in1R9          (PE, replicates 9 rows -> 81, fp32r)
    Z      = in1rep * in2rep      (DVE tensor_tensor, in2 broadcast over c)
    outps  = T_all.T @ Z          (PE, fp32r, block-diagonal CG matrix)
    copy PSUM->SBUF (ScalarE), DMA store in [81, B_LOC*32] layout.
  Host un-permutes the [q, (t, c, b)] device layout into [b, o] (pure data
  movement). T_all / S and the output column permutation are derived from the
  COO tables (cb_vals, i1_idx, i2_idx, out_idx) passed in as inputs.
"""

from contextlib import ExitStack

import numpy as np

import concourse.bass as bass
import concourse.mybir as mybir
import concourse.tile as tile
from concourse import bacc
from concourse.bass_utils import run_bass_kernel_spmd

# ---- hardcoded problem geometry ----
B = 8192
DIM1 = 288
DIM2 = 9
CBH = 2592
NCORES = 8
BLOC = B // NCORES          # 1024 batch rows per core
PT = 128                    # partition tile (batch rows per tile)
NT = BLOC // PT             # 8 tiles per core
NCHAN = 32
NROW = 81                   # (g, m1, j) product rows
NQ = 81                     # (g, o3) output rows per channel
CHUNK_C = 4                 # channels per 512-col chunk
NCHUNK = NCHAN // CHUNK_C   # 8 chunks per batch tile
FREE = CHUNK_C * PT         # 512 = matmul moving-dim per chunk

# per group: (col offset in in1, 2*l1+1, gm1 row offset, q offset, D3)
GRP = [(0, 1, 0, 0, 9), (32, 3, 1, 9, 27), (128, 5, 4, 36, 45)]

F32 = mybir.dt.float32
F32R = mybir.dt.float32r

_cache: dict = {}


# --------------------------------------------------------------------------
# Tables from the COO inputs
# --------------------------------------------------------------------------
def _build_tables(cb_vals, i1_idx, i2_idx, out_idx):
    """Build T_all [81, 81], S [9, 81], colmap [81, 32] from the COO triple.

    T_all[p, q]: coefficient taking product row p=(g,m1,j) to output row
    q=(g,o3).  colmap[q, c]: the out column for output row q of channel c.
    Relies on (and verifies) the CG coefficients being channel-independent
    and the out-column order being consistent across channels.
    """
    cb = np.asarray(cb_vals, np.float64)
    i1 = np.asarray(i1_idx, np.int64)
    i2 = np.asarray(i2_idx, np.int64)
    oo = np.asarray(out_idx, np.int64)

    g = np.where(i1 < 32, 0, np.where(i1 < 128, 1, 2))
    rel = i1 - np.array([0, 32, 128])[g]
    width = np.array([1, 3, 5])[g]
    c = rel // width
    m1 = rel % width
    gm1 = np.array([0, 1, 4])[g] + m1
    p = gm1 * 9 + i2

    # distinct out columns per (g, c), sorted ascending -> rank k
    ocols: dict = {}
    for gg, cc, o in zip(g, c, oo):
        ocols.setdefault((int(gg), int(cc)), set()).add(int(o))
    rank: dict = {}
    for (gg, cc), s in ocols.items():
        d3 = GRP[gg][4]
        assert len(s) == d3, f"group {gg} chan {cc}: {len(s)} cols != {d3}"
        for k, o in enumerate(sorted(s)):
            rank[(gg, cc, o)] = k

    T_all = np.zeros((NROW, NQ), np.float64)
    have = np.zeros((NROW, NQ), bool)
    colmap = -np.ones((NQ, NCHAN), np.int64)
    for n in range(len(cb)):
        gg, cc = int(g[n]), int(c[n])
        q = GRP[gg][3] + rank[(gg, cc, int(oo[n]))]
        colmap[q, cc] = oo[n]
        if have[p[n], q]:
            assert abs(T_all[p[n], q] - cb[n]) < 1e-5, "CG not channel-uniform"
        else:
            T_all[p[n], q] = cb[n]
            have[p[n], q] = True
    assert (colmap >= 0).all()
    perm = colmap.reshape(-1)
    assert np.array_equal(np.sort(perm), np.arange(CBH)), "colmap not a perm"

    S = np.zeros((9, NROW), np.float32)
    for pp in range(NROW):
        S[pp // 9, pp] = 1.0
    return T_all.astype(np.float32), S, perm


# --------------------------------------------------------------------------
# Device kernel
# --------------------------------------------------------------------------
def _trace_module():
    nc = bacc.Bacc(trn_type="TRN2")
    in1r = nc.dram_tensor("in1r", [9, NT * NCHAN * PT], F32R, kind="ExternalInput")
    in2r = nc.dram_tensor("in2r", [NROW, BLOC], F32, kind="ExternalInput")
    tmat = nc.dram_tensor("tmat", [NROW, NQ], F32R, kind="ExternalInput")
    smat = nc.dram_tensor("smat", [9, NROW], F32R, kind="ExternalInput")
    outd = nc.dram_tensor("outd", [NQ, NT * NCHAN * PT], F32, kind="ExternalOutput")

    with tile.TileContext(nc) as tc, ExitStack() as ctx:
        _cg_body(ctx, tc, outd, in1r, in2r, tmat, smat)
    nc.compile()
    return nc


GROUPW = 2
FUSE = True
PSA_BUFS = 2
PSB_BUFS = 2
ZP_BUFS = 3


def _cg_body(ctx, tc, outd, in1r, in2r, tmat, smat):
    nc = tc.nc
    const = ctx.enter_context(tc.tile_pool(name="const", bufs=1))
    inp = ctx.enter_context(tc.tile_pool(name="inp", bufs=3))
    psa = ctx.enter_context(tc.tile_pool(name="psa", bufs=PSA_BUFS, space="PSUM"))
    psb = ctx.enter_context(tc.tile_pool(name="psb", bufs=PSB_BUFS, space="PSUM"))
    zp = ctx.enter_context(tc.tile_pool(name="zp", bufs=ZP_BUFS))
    op = ctx.enter_context(tc.tile_pool(name="op", bufs=2))

    # gm1-major so matmul rhs slices sit at base partition 0.
    in1v = in1r.ap().rearrange("g (t c b) -> g t c b", t=NT, c=NCHAN)
    sb_in2 = const.tile([NROW, BLOC], F32)
    nc.sync.dma_start(out=sb_in2, in_=in2r.ap())
    sb_t = const.tile([NROW, NQ], F32R)
    nc.sync.dma_start(out=sb_t, in_=tmat.ap())
    sb_s = const.tile([9, NROW], F32R)
    nc.sync.dma_start(out=sb_s, in_=smat.ap())

    lhs_s = sb_s[:]
    lhs_t = sb_t[:]

    for t in range(NT):
        in1t = inp.tile([9, NCHAN, PT], F32R)
        nc.sync.dma_start(out=in1t, in_=in1v[:, t])
        out_sb = op.tile([NQ, NCHUNK, CHUNK_C, PT], F32)
        for c0 in range(0, NCHUNK, GROUPW):
            # Pair-fused: two 512-col matmuls share one 2-bank PSUM tile so
            # the DVE multiply and ACT copy run once per pair at 1024 wide,
            # amortizing their fixed per-op overheads.
            pa = psa.tile([NROW, GROUPW, CHUNK_C, PT], F32)
            for i in range(GROUPW):
                cc = c0 + i
                nc.tensor.matmul(
                    pa[:, i],
                    lhsT=lhs_s,
                    rhs=in1t[:, cc * CHUNK_C:(cc + 1) * CHUNK_C, :],
                    start=True,
                    stop=True,
                )
            z = zp.tile([NROW, GROUPW, CHUNK_C, PT], F32R)
            in2bb = (
                sb_in2[:, t * PT:(t + 1) * PT]
                .unsqueeze(1)
                .unsqueeze(1)
                .broadcast_to((NROW, GROUPW, CHUNK_C, PT))
            )
            nc.vector.tensor_tensor(
                out=z[:], in0=pa[:], in1=in2bb, op=mybir.AluOpType.mult
            )
            pb = psb.tile([NROW, GROUPW, CHUNK_C, PT], F32)
            for i in range(GROUPW):
                nc.tensor.matmul(
                    pb[:, i],
                    lhsT=lhs_t,
                    rhs=z[:, i],
                    start=True,
                    stop=True,
                )
            nc.scalar.copy(
                out=out_sb[:, c0:c0 + GROUPW], in_=pb[:]
            )
        nc.sync.dma_start(
            out=outd.ap()[:, t * NCHAN * PT:(t + 1) * NCHAN * PT], in_=out_sb
        )


def _get_module():
    if "nc" not in _cache:
        _cache["nc"] = _trace_module()
    return _cache["nc"]


# --------------------------------------------------------------------------
# Host glue
# --------------------------------------------------------------------------
def _prep_in1(in1):
    """in1 [B, 288] -> per-core [NT*9, 32*128]: row (t, gm1), free (c, b)."""
    g0 = in1[:, 0:32].T[None]                                  # [1, 32, B]
    g1 = in1[:, 32:128].reshape(B, 32, 3).transpose(2, 1, 0)   # [3, 32, B]
    g2 = in1[:, 128:288].reshape(B, 32, 5).transpose(2, 1, 0)  # [5, 32, B]
    r = np.concatenate([g0, g1, g2], axis=0)                   # [9, 32, B]
    cores = []
    for k in range(NCORES):
        rc = r[:, :, k * BLOC:(k + 1) * BLOC].reshape(9, NCHAN, NT, PT)
        rc = rc.transpose(0, 2, 1, 3).reshape(9, NT * NCHAN * PT)
        cores.append(np.ascontiguousarray(rc, np.float32))
    return cores


def _prep_in2(in2):
    rep = in2.T[np.arange(NROW) % 9]                           # [81, B]
    return [
        np.ascontiguousarray(rep[:, k * BLOC:(k + 1) * BLOC], np.float32)
        for k in range(NCORES)
    ]


def kernel(in1, in2, cb_vals, i1_idx, i2_idx, out_idx, **run_kwargs):
    in1 = np.asarray(in1, np.float32)
    in2 = np.asarray(in2, np.float32)
    assert in1.shape == (B, DIM1) and in2.shape == (B, DIM2)

    if "tables" not in _cache:
        _cache["tables"] = _build_tables(cb_vals, i1_idx, i2_idx, out_idx)
    t_all, s_mat, perm = _cache["tables"]

    nc = _get_module()
    in1_cores = _prep_in1(in1)
    in2_cores = _prep_in2(in2)
    in_maps = [
        {"in1r": in1_cores[k], "in2r": in2_cores[k], "tmat": t_all, "smat": s_mat}
        for k in range(NCORES)
    ]
    res = run_bass_kernel_spmd(nc, in_maps, core_ids=list(range(NCORES)), **run_kwargs)
    _cache["last_results"] = res

    out = np.empty((B, CBH), np.float32)
    for k in range(NCORES):
        od = res.results[k]["outd"].reshape(NQ, NT, NCHAN, PT)
        oc = od.transpose(1, 3, 0, 2).reshape(BLOC, NQ * NCHAN)
        out[k * BLOC:(k + 1) * BLOC, perm] = oc
    return out



# revision 5
# speedup vs baseline: 1.2519x; 1.2519x over previous
"""Trainium2 Bass kernel: e3nn-style CudaTensorProduct (CG tensor product).

out[b, o] = sum_nnz cb * in1[b, i1] * in2[b, i2]

in1: [8192, 288] = 32 channels each of l1=0,1,2 (dims 1/3/5)
in2: [8192, 9]   = spherical harmonics l2=0..2
out: [8192, 2592]

Device formulation (per core, batch slice of 1024, data parallel over 8 cores):
  For each group g (l1=0,1,2) the CG coefficients are channel-independent.
  Let p = (g, m1, j) index the 81 = 9*(1+3+5) "product rows" and
  q = (g, o3) index the 81 = 9+27+45 output rows per channel.
    Z[p, (c, b)]   = in1[b, col(g,c,m1)] * in2[b, j]
    out[q, (c, b)] = sum_p T_all[p, q] * Z[p, (c, b)]
  On device:
    in1rep = S.T @ in1 (bf16)     (PE, replicates 9 rows -> 81)
    Z      = in1rep * in2rep      (DVE tensor_tensor, f32 PSUM -> f32r SBUF)
    outps  = T_all.T @ Z          (PE, f32r; neuronxcc rejects mixed-width)
    copy PSUM->SBUF as bf16 (ACT), DMA store in [81, B_LOC*32] layout.
  bf16 halves the dominant output-store DMA traffic (error budget 2e-2 »
  bf16 rounding).  The last batch tile fans its evacuations across ACT/DVE
  (GPSIMD cannot access PSUM) and stores per chunk-pair to shorten the
  pipeline drain.
  Host un-permutes the [q, (t, c, b)] device layout into [b, o] (pure data
  movement + dtype cast). T_all / S and the output column permutation are
  derived from the COO tables (cb_vals, i1_idx, i2_idx, out_idx) inputs.
"""

from contextlib import ExitStack

import ml_dtypes
import numpy as np

import concourse.bass as bass
import concourse.mybir as mybir
import concourse.tile as tile
from concourse import bacc
from concourse.bass_utils import run_bass_kernel_spmd

# ---- hardcoded problem geometry ----
B = 8192
DIM1 = 288
DIM2 = 9
CBH = 2592
NCORES = 8
BLOC = B // NCORES          # 1024 batch rows per core
PT = 128                    # partition tile (batch rows per tile)
NT = BLOC // PT             # 8 tiles per core
NCHAN = 32
NROW = 81                   # (g, m1, j) product rows
NQ = 81                     # (g, o3) output rows per channel
CHUNK_C = 4                 # channels per 512-col chunk
NCHUNK = NCHAN // CHUNK_C   # 8 chunks per batch tile
FREE = CHUNK_C * PT         # 512 = matmul moving-dim per chunk

# per group: (col offset in in1, 2*l1+1, gm1 row offset, q offset, D3)
GRP = [(0, 1, 0, 0, 9), (32, 3, 1, 9, 27), (128, 5, 4, 36, 45)]

F32 = mybir.dt.float32
F32R = mybir.dt.float32r
BF16 = mybir.dt.bfloat16
NPBF16 = ml_dtypes.bfloat16

_cache: dict = {}


# --------------------------------------------------------------------------
# Tables from the COO inputs
# --------------------------------------------------------------------------
def _build_tables(cb_vals, i1_idx, i2_idx, out_idx):
    """Build T_all [81, 81], S [9, 81], colmap [81, 32] from the COO triple.

    T_all[p, q]: coefficient taking product row p=(g,m1,j) to output row
    q=(g,o3).  colmap[q, c]: the out column for output row q of channel c.
    Relies on (and verifies) the CG coefficients being channel-independent
    and the out-column order being consistent across channels.
    """
    cb = np.asarray(cb_vals, np.float64)
    i1 = np.asarray(i1_idx, np.int64)
    i2 = np.asarray(i2_idx, np.int64)
    oo = np.asarray(out_idx, np.int64)

    g = np.where(i1 < 32, 0, np.where(i1 < 128, 1, 2))
    rel = i1 - np.array([0, 32, 128])[g]
    width = np.array([1, 3, 5])[g]
    c = rel // width
    m1 = rel % width
    gm1 = np.array([0, 1, 4])[g] + m1
    p = gm1 * 9 + i2

    # distinct out columns per (g, c), sorted ascending -> rank k
    ocols: dict = {}
    for gg, cc, o in zip(g, c, oo):
        ocols.setdefault((int(gg), int(cc)), set()).add(int(o))
    rank: dict = {}
    for (gg, cc), s in ocols.items():
        d3 = GRP[gg][4]
        assert len(s) == d3, f"group {gg} chan {cc}: {len(s)} cols != {d3}"
        for k, o in enumerate(sorted(s)):
            rank[(gg, cc, o)] = k

    T_all = np.zeros((NROW, NQ), np.float64)
    have = np.zeros((NROW, NQ), bool)
    colmap = -np.ones((NQ, NCHAN), np.int64)
    for n in range(len(cb)):
        gg, cc = int(g[n]), int(c[n])
        q = GRP[gg][3] + rank[(gg, cc, int(oo[n]))]
        colmap[q, cc] = oo[n]
        if have[p[n], q]:
            assert abs(T_all[p[n], q] - cb[n]) < 1e-5, "CG not channel-uniform"
        else:
            T_all[p[n], q] = cb[n]
            have[p[n], q] = True
    assert (colmap >= 0).all()
    perm = colmap.reshape(-1)
    assert np.array_equal(np.sort(perm), np.arange(CBH)), "colmap not a perm"

    S = np.zeros((9, NROW), np.float32)
    for pp in range(NROW):
        S[pp // 9, pp] = 1.0
    return T_all.astype(np.float32), S, perm


# --------------------------------------------------------------------------
# Device kernel
# --------------------------------------------------------------------------
def _trace_module():
    nc = bacc.Bacc(trn_type="TRN2")
    in1r = nc.dram_tensor("in1r", [9, NT * NCHAN * PT], BF16, kind="ExternalInput")
    in2r = nc.dram_tensor("in2r", [NROW, BLOC], F32, kind="ExternalInput")
    tmat = nc.dram_tensor("tmat", [NROW, NQ], F32R, kind="ExternalInput")
    smat = nc.dram_tensor("smat", [9, NROW], BF16, kind="ExternalInput")
    outd = nc.dram_tensor("outd", [NQ, NT * NCHAN * PT], BF16, kind="ExternalOutput")

    with tile.TileContext(nc) as tc, ExitStack() as ctx:
        _cg_body(ctx, tc, outd, in1r, in2r, tmat, smat)
    nc.compile()
    return nc


GROUPW = 2                  # chunks per pair-unit
NPAIR = NCHUNK // GROUPW    # 4 pair-units per tile
PSA_BUFS = 2
PSB_BUFS = 2
ZP_BUFS = 4
INP_BUFS = 4
OP_BUFS = 3


def _cg_body(ctx, tc, outd, in1r, in2r, tmat, smat):
    nc = tc.nc
    const = ctx.enter_context(tc.tile_pool(name="const", bufs=1))
    inp = ctx.enter_context(tc.tile_pool(name="inp", bufs=INP_BUFS))
    psa = ctx.enter_context(tc.tile_pool(name="psa", bufs=PSA_BUFS, space="PSUM"))
    psb = ctx.enter_context(tc.tile_pool(name="psb", bufs=PSB_BUFS, space="PSUM"))
    zp = ctx.enter_context(tc.tile_pool(name="zp", bufs=ZP_BUFS))
    op = ctx.enter_context(tc.tile_pool(name="op", bufs=OP_BUFS))

    # gm1-major so matmul rhs slices sit at base partition 0.
    in1v = in1r.ap().rearrange("g (t c b) -> g t c b", t=NT, c=NCHAN)
    sb_in2 = const.tile([NROW, BLOC], F32)
    nc.sync.dma_start(out=sb_in2, in_=in2r.ap())
    sb_t = const.tile([NROW, NQ], F32R)
    nc.sync.dma_start(out=sb_t, in_=tmat.ap())
    sb_s = const.tile([9, NROW], BF16)
    nc.sync.dma_start(out=sb_s, in_=smat.ap())

    lhs_s = sb_s[:]
    lhs_t = sb_t[:]

    for t in range(NT):
        in1t = inp.tile([9, NCHAN, PT], BF16)
        nc.sync.dma_start(out=in1t, in_=in1v[:, t])
        out_sb = op.tile([NQ, NCHUNK, CHUNK_C, PT], BF16)
        last_tile = t == NT - 1
        for pi in range(NPAIR):
            c0 = pi * GROUPW
            # Pair-fused: two 512-col matmuls share one 2-bank PSUM tile so
            # the multiply and evacuation run once per pair at 1024 wide,
            # amortizing their fixed per-op overheads.
            pa = psa.tile([NROW, GROUPW, CHUNK_C, PT], F32)
            for i in range(GROUPW):
                cc = c0 + i
                nc.tensor.matmul(
                    pa[:, i],
                    lhsT=lhs_s,
                    rhs=in1t[:, cc * CHUNK_C:(cc + 1) * CHUNK_C, :],
                    start=True,
                    stop=True,
                )
            z = zp.tile([NROW, GROUPW, CHUNK_C, PT], F32R)
            in2bb = (
                sb_in2[:, t * PT:(t + 1) * PT]
                .unsqueeze(1)
                .unsqueeze(1)
                .broadcast_to((NROW, GROUPW, CHUNK_C, PT))
            )
            nc.vector.tensor_tensor(
                out=z[:], in0=pa[:], in1=in2bb, op=mybir.AluOpType.mult
            )
            pb = psb.tile([NROW, GROUPW, CHUNK_C, PT], F32)
            for i in range(GROUPW):
                nc.tensor.matmul(
                    pb[:, i],
                    lhsT=lhs_t,
                    rhs=z[:, i],
                    start=True,
                    stop=True,
                )
            if last_tile:
                # Fan the final tile's evacuations across engines and store
                # per pair to shorten the pipeline drain.
                evac = [nc.scalar.copy, nc.vector.tensor_copy][pi % 2]
                evac(out=out_sb[:, c0:c0 + GROUPW], in_=pb[:])
                nc.sync.dma_start(
                    out=outd.ap()[:, (t * NCHAN + c0 * CHUNK_C) * PT:
                                  (t * NCHAN + (c0 + GROUPW) * CHUNK_C) * PT],
                    in_=out_sb[:, c0:c0 + GROUPW],
                )
            else:
                nc.scalar.copy(out=out_sb[:, c0:c0 + GROUPW], in_=pb[:])
        if not last_tile:
            nc.sync.dma_start(
                out=outd.ap()[:, t * NCHAN * PT:(t + 1) * NCHAN * PT], in_=out_sb
            )


def _get_module():
    if "nc" not in _cache:
        _cache["nc"] = _trace_module()
    return _cache["nc"]


# --------------------------------------------------------------------------
# Host glue
# --------------------------------------------------------------------------
def _prep_in1(in1):
    """in1 [B, 288] -> per-core bf16 [9, NT*32*128]: row gm1, free (t, c, b)."""
    g0 = in1[:, 0:32].T[None]                                  # [1, 32, B]
    g1 = in1[:, 32:128].reshape(B, 32, 3).transpose(2, 1, 0)   # [3, 32, B]
    g2 = in1[:, 128:288].reshape(B, 32, 5).transpose(2, 1, 0)  # [5, 32, B]
    r = np.concatenate([g0, g1, g2], axis=0)                   # [9, 32, B]
    cores = []
    for k in range(NCORES):
        rc = r[:, :, k * BLOC:(k + 1) * BLOC].reshape(9, NCHAN, NT, PT)
        rc = rc.transpose(0, 2, 1, 3).reshape(9, NT * NCHAN * PT)
        cores.append(np.ascontiguousarray(rc.astype(NPBF16)))
    return cores


def _prep_in2(in2):
    rep = in2.T[np.arange(NROW) % 9]                           # [81, B]
    return [
        np.ascontiguousarray(rep[:, k * BLOC:(k + 1) * BLOC], np.float32)
        for k in range(NCORES)
    ]


def kernel(in1, in2, cb_vals, i1_idx, i2_idx, out_idx, **run_kwargs):
    in1 = np.asarray(in1, np.float32)
    in2 = np.asarray(in2, np.float32)
    assert in1.shape == (B, DIM1) and in2.shape == (B, DIM2)

    if "tables" not in _cache:
        _cache["tables"] = _build_tables(cb_vals, i1_idx, i2_idx, out_idx)
    t_all, s_mat, perm = _cache["tables"]

    nc = _get_module()
    in1_cores = _prep_in1(in1)
    in2_cores = _prep_in2(in2)
    s_bf = s_mat.astype(NPBF16)
    in_maps = [
        {"in1r": in1_cores[k], "in2r": in2_cores[k], "tmat": t_all, "smat": s_bf}
        for k in range(NCORES)
    ]
    res = run_bass_kernel_spmd(nc, in_maps, core_ids=list(range(NCORES)), **run_kwargs)
    _cache["last_results"] = res

    out = np.empty((B, CBH), np.float32)
    for k in range(NCORES):
        od = np.asarray(res.results[k]["outd"]).astype(np.float32)
        od = od.reshape(NQ, NT, NCHAN, PT)
        oc = od.transpose(1, 3, 0, 2).reshape(BLOC, NQ * NCHAN)
        out[k * BLOC:(k + 1) * BLOC, perm] = oc
    return out


# revision 6
# speedup vs baseline: 1.2915x; 1.0317x over previous
"""Trainium2 Bass kernel: e3nn-style CudaTensorProduct (CG tensor product).

out[b, o] = sum_nnz cb * in1[b, i1] * in2[b, i2]

in1: [8192, 288] = 32 channels each of l1=0,1,2 (dims 1/3/5)
in2: [8192, 9]   = spherical harmonics l2=0..2
out: [8192, 2592]

Device formulation (per core, batch slice of 1024, data parallel over 8 cores):
  For each group g (l1=0,1,2) the CG coefficients are channel-independent.
  Let p = (g, m1, j) index the 81 = 9*(1+3+5) "product rows" and
  q = (g, o3) index the 81 = 9+27+45 output rows per channel.
    Z[p, (c, b)]   = in1[b, col(g,c,m1)] * in2[b, j]
    out[q, (c, b)] = sum_p T_all[p, q] * Z[p, (c, b)]
  On device:
    in1rep = S.T @ in1 (bf16)     (PE, replicates 9 rows -> 81)
    Z      = in1rep * in2rep      (DVE tensor_tensor, f32 PSUM -> f32r SBUF)
    outps  = T_all.T @ Z          (PE, f32r; neuronxcc rejects mixed-width)
    copy PSUM->SBUF as bf16 (ACT), DMA store in [81, B_LOC*32] layout.
  bf16 halves the dominant output-store DMA traffic (error budget 2e-2 »
  bf16 rounding).  The last batch tile fans its evacuations across ACT/DVE
  (GPSIMD cannot access PSUM) and stores per chunk-pair to shorten the
  pipeline drain.
  Host un-permutes the [q, (t, c, b)] device layout into [b, o] (pure data
  movement + dtype cast). T_all / S and the output column permutation are
  derived from the COO tables (cb_vals, i1_idx, i2_idx, out_idx) inputs.
"""

from contextlib import ExitStack

import ml_dtypes
import numpy as np

import concourse.bass as bass
import concourse.mybir as mybir
import concourse.tile as tile
from concourse import bacc
from concourse.bass_utils import run_bass_kernel_spmd

# ---- hardcoded problem geometry ----
B = 8192
DIM1 = 288
DIM2 = 9
CBH = 2592
NCORES = 8
BLOC = B // NCORES          # 1024 batch rows per core
PT = 128                    # partition tile (batch rows per tile)
NT = BLOC // PT             # 8 tiles per core
NCHAN = 32
NROW = 81                   # (g, m1, j) product rows
NQ = 81                     # (g, o3) output rows per channel
CHUNK_C = 4                 # channels per 512-col chunk
NCHUNK = NCHAN // CHUNK_C   # 8 chunks per batch tile
FREE = CHUNK_C * PT         # 512 = matmul moving-dim per chunk

# per group: (col offset in in1, 2*l1+1, gm1 row offset, q offset, D3)
GRP = [(0, 1, 0, 0, 9), (32, 3, 1, 9, 27), (128, 5, 4, 36, 45)]

F32 = mybir.dt.float32
F32R = mybir.dt.float32r
BF16 = mybir.dt.bfloat16
NPBF16 = ml_dtypes.bfloat16

_cache: dict = {}


# --------------------------------------------------------------------------
# Tables from the COO inputs
# --------------------------------------------------------------------------
def _build_tables(cb_vals, i1_idx, i2_idx, out_idx):
    """Build T_all [81, 81], S [9, 81], colmap [81, 32] from the COO triple.

    T_all[p, q]: coefficient taking product row p=(g,m1,j) to output row
    q=(g,o3).  colmap[q, c]: the out column for output row q of channel c.
    Relies on (and verifies) the CG coefficients being channel-independent
    and the out-column order being consistent across channels.
    """
    cb = np.asarray(cb_vals, np.float64)
    i1 = np.asarray(i1_idx, np.int64)
    i2 = np.asarray(i2_idx, np.int64)
    oo = np.asarray(out_idx, np.int64)

    g = np.where(i1 < 32, 0, np.where(i1 < 128, 1, 2))
    rel = i1 - np.array([0, 32, 128])[g]
    width = np.array([1, 3, 5])[g]
    c = rel // width
    m1 = rel % width
    gm1 = np.array([0, 1, 4])[g] + m1
    p = gm1 * 9 + i2

    # distinct out columns per (g, c), sorted ascending -> rank k
    ocols: dict = {}
    for gg, cc, o in zip(g, c, oo):
        ocols.setdefault((int(gg), int(cc)), set()).add(int(o))
    rank: dict = {}
    for (gg, cc), s in ocols.items():
        d3 = GRP[gg][4]
        assert len(s) == d3, f"group {gg} chan {cc}: {len(s)} cols != {d3}"
        for k, o in enumerate(sorted(s)):
            rank[(gg, cc, o)] = k

    T_all = np.zeros((NROW, NQ), np.float64)
    have = np.zeros((NROW, NQ), bool)
    colmap = -np.ones((NQ, NCHAN), np.int64)
    for n in range(len(cb)):
        gg, cc = int(g[n]), int(c[n])
        q = GRP[gg][3] + rank[(gg, cc, int(oo[n]))]
        colmap[q, cc] = oo[n]
        if have[p[n], q]:
            assert abs(T_all[p[n], q] - cb[n]) < 1e-5, "CG not channel-uniform"
        else:
            T_all[p[n], q] = cb[n]
            have[p[n], q] = True
    assert (colmap >= 0).all()
    perm = colmap.reshape(-1)
    assert np.array_equal(np.sort(perm), np.arange(CBH)), "colmap not a perm"

    S = np.zeros((9, NROW), np.float32)
    for pp in range(NROW):
        S[pp // 9, pp] = 1.0
    return T_all.astype(np.float32), S, perm


# --------------------------------------------------------------------------
# Device kernel
# --------------------------------------------------------------------------
def _trace_module():
    nc = bacc.Bacc(trn_type="TRN2")
    in1r = nc.dram_tensor("in1r", [9, NT * NCHAN * PT], BF16, kind="ExternalInput")
    in2r = nc.dram_tensor("in2r", [NROW, BLOC], F32, kind="ExternalInput")
    tmat = nc.dram_tensor("tmat", [NROW, NQ], F32R, kind="ExternalInput")
    smat = nc.dram_tensor("smat", [9, NROW], BF16, kind="ExternalInput")
    outd = nc.dram_tensor("outd", [NQ, NT * NCHAN * PT], BF16, kind="ExternalOutput")

    with tile.TileContext(nc) as tc, ExitStack() as ctx:
        _cg_body(ctx, tc, outd, in1r, in2r, tmat, smat)
    nc.compile()
    return nc


GROUPW = 2                  # chunks per pair-unit
NPAIR = NCHUNK // GROUPW    # 4 pair-units per tile
PSA_BUFS = 2
PSB_BUFS = 2
ZP_BUFS = 4
INP_BUFS = 4
OP_BUFS = 3


def _cg_body(ctx, tc, outd, in1r, in2r, tmat, smat):
    nc = tc.nc
    const = ctx.enter_context(tc.tile_pool(name="const", bufs=1))
    inp = ctx.enter_context(tc.tile_pool(name="inp", bufs=INP_BUFS))
    psa = ctx.enter_context(tc.tile_pool(name="psa", bufs=PSA_BUFS, space="PSUM"))
    psb = ctx.enter_context(tc.tile_pool(name="psb", bufs=PSB_BUFS, space="PSUM"))
    zp = ctx.enter_context(tc.tile_pool(name="zp", bufs=ZP_BUFS))
    op = ctx.enter_context(tc.tile_pool(name="op", bufs=OP_BUFS))

    # gm1-major so matmul rhs slices sit at base partition 0.
    in1v = in1r.ap().rearrange("g (t c b) -> g t c b", t=NT, c=NCHAN)
    sb_in2 = const.tile([NROW, BLOC], F32)
    nc.sync.dma_start(out=sb_in2, in_=in2r.ap())
    sb_t = const.tile([NROW, NQ], F32R)
    nc.sync.dma_start(out=sb_t, in_=tmat.ap())
    sb_s = const.tile([9, NROW], BF16)
    nc.sync.dma_start(out=sb_s, in_=smat.ap())

    lhs_s = sb_s[:]
    lhs_t = sb_t[:]

    for t in range(NT):
        in1t = inp.tile([9, NCHAN, PT], BF16)
        nc.sync.dma_start(out=in1t, in_=in1v[:, t])
        out_sb = op.tile([NQ, NCHUNK, CHUNK_C, PT], BF16)
        last_tile = t == NT - 1
        for pi in range(NPAIR):
            c0 = pi * GROUPW
            # Pair-fused: two 512-col matmuls share one 2-bank PSUM tile so
            # the multiply and evacuation run once per pair at 1024 wide,
            # amortizing their fixed per-op overheads.
            pa = psa.tile([NROW, GROUPW, CHUNK_C, PT], F32)
            for i in range(GROUPW):
                cc = c0 + i
                nc.tensor.matmul(
                    pa[:, i],
                    lhsT=lhs_s,
                    rhs=in1t[:, cc * CHUNK_C:(cc + 1) * CHUNK_C, :],
                    start=True,
                    stop=True,
                )
            z = zp.tile([NROW, GROUPW, CHUNK_C, PT], F32R)
            in2bb = (
                sb_in2[:, t * PT:(t + 1) * PT]
                .unsqueeze(1)
                .unsqueeze(1)
                .broadcast_to((NROW, GROUPW, CHUNK_C, PT))
            )
            nc.vector.tensor_tensor(
                out=z[:], in0=pa[:], in1=in2bb, op=mybir.AluOpType.mult
            )
            pb = psb.tile([NROW, GROUPW, CHUNK_C, PT], F32)
            for i in range(GROUPW):
                nc.tensor.matmul(
                    pb[:, i],
                    lhsT=lhs_t,
                    rhs=z[:, i],
                    start=True,
                    stop=True,
                )
            if last_tile:
                # Fan the final tile's evacuations across engines and store
                # per pair to shorten the pipeline drain.
                evac = [nc.scalar.copy, nc.vector.tensor_copy][pi % 2]
                evac(out=out_sb[:, c0:c0 + GROUPW], in_=pb[:])
                nc.gpsimd.dma_start(
                    out=outd.ap()[:, (t * NCHAN + c0 * CHUNK_C) * PT:
                                  (t * NCHAN + (c0 + GROUPW) * CHUNK_C) * PT],
                    in_=out_sb[:, c0:c0 + GROUPW],
                )
            else:
                nc.scalar.copy(out=out_sb[:, c0:c0 + GROUPW], in_=pb[:])
        if not last_tile:
            # Stores go out on Pool's SWDGE queue so they never queue ahead
            # of the SP-issued in1 loads on the shared DMA dispatch order.
            nc.gpsimd.dma_start(
                out=outd.ap()[:, t * NCHAN * PT:(t + 1) * NCHAN * PT], in_=out_sb
            )


def _get_module():
    if "nc" not in _cache:
        _cache["nc"] = _trace_module()
    return _cache["nc"]


# --------------------------------------------------------------------------
# Host glue
# --------------------------------------------------------------------------
def _prep_in1(in1):
    """in1 [B, 288] -> per-core bf16 [9, NT*32*128]: row gm1, free (t, c, b)."""
    g0 = in1[:, 0:32].T[None]                                  # [1, 32, B]
    g1 = in1[:, 32:128].reshape(B, 32, 3).transpose(2, 1, 0)   # [3, 32, B]
    g2 = in1[:, 128:288].reshape(B, 32, 5).transpose(2, 1, 0)  # [5, 32, B]
    r = np.concatenate([g0, g1, g2], axis=0)                   # [9, 32, B]
    cores = []
    for k in range(NCORES):
        rc = r[:, :, k * BLOC:(k + 1) * BLOC].reshape(9, NCHAN, NT, PT)
        rc = rc.transpose(0, 2, 1, 3).reshape(9, NT * NCHAN * PT)
        cores.append(np.ascontiguousarray(rc.astype(NPBF16)))
    return cores


def _prep_in2(in2):
    rep = in2.T[np.arange(NROW) % 9]                           # [81, B]
    return [
        np.ascontiguousarray(rep[:, k * BLOC:(k + 1) * BLOC], np.float32)
        for k in range(NCORES)
    ]


def kernel(in1, in2, cb_vals, i1_idx, i2_idx, out_idx, **run_kwargs):
    in1 = np.asarray(in1, np.float32)
    in2 = np.asarray(in2, np.float32)
    assert in1.shape == (B, DIM1) and in2.shape == (B, DIM2)

    if "tables" not in _cache:
        _cache["tables"] = _build_tables(cb_vals, i1_idx, i2_idx, out_idx)
    t_all, s_mat, perm = _cache["tables"]

    nc = _get_module()
    in1_cores = _prep_in1(in1)
    in2_cores = _prep_in2(in2)
    s_bf = s_mat.astype(NPBF16)
    in_maps = [
        {"in1r": in1_cores[k], "in2r": in2_cores[k], "tmat": t_all, "smat": s_bf}
        for k in range(NCORES)
    ]
    res = run_bass_kernel_spmd(nc, in_maps, core_ids=list(range(NCORES)), **run_kwargs)
    _cache["last_results"] = res

    out = np.empty((B, CBH), np.float32)
    for k in range(NCORES):
        od = np.asarray(res.results[k]["outd"]).astype(np.float32)
        od = od.reshape(NQ, NT, NCHAN, PT)
        oc = od.transpose(1, 3, 0, 2).reshape(BLOC, NQ * NCHAN)
        out[k * BLOC:(k + 1) * BLOC, perm] = oc
    return out


# revision 9
# speedup vs baseline: 1.2928x; 1.0010x over previous
"""Trainium2 Bass kernel: e3nn-style CudaTensorProduct (CG tensor product).

out[b, o] = sum_nnz cb * in1[b, i1] * in2[b, i2]

in1: [8192, 288] = 32 channels each of l1=0,1,2 (dims 1/3/5)
in2: [8192, 9]   = spherical harmonics l2=0..2
out: [8192, 2592]

Device formulation (per core, batch slice of 1024, data parallel over 8 cores):
  For each group g (l1=0,1,2) the CG coefficients are channel-independent.
  Let p = (g, m1, j) index the 81 = 9*(1+3+5) "product rows" and
  q = (g, o3) index the 81 = 9+27+45 output rows per channel.
    Z[p, (c, b)]   = in1[b, col(g,c,m1)] * in2[b, j]
    out[q, (c, b)] = sum_p T_all[p, q] * Z[p, (c, b)]
  On device:
    in1rep = S.T @ in1 (bf16)     (PE, replicates 9 rows -> 81)
    Z      = in1rep * in2rep      (DVE tensor_tensor, f32 PSUM -> f32r SBUF)
    outps  = T_all.T @ Z          (PE, f32r; neuronxcc rejects mixed-width)
    copy PSUM->SBUF as bf16 (ACT), DMA store in [81, B_LOC*32] layout.
  bf16 halves the dominant output-store DMA traffic (error budget 2e-2 »
  bf16 rounding).  The last batch tile fans its evacuations across ACT/DVE
  (GPSIMD cannot access PSUM) and stores per chunk-pair to shorten the
  pipeline drain.
  Host un-permutes the [q, (t, c, b)] device layout into [b, o] (pure data
  movement + dtype cast). T_all / S and the output column permutation are
  derived from the COO tables (cb_vals, i1_idx, i2_idx, out_idx) inputs.
"""

from contextlib import ExitStack

import ml_dtypes
import numpy as np

import concourse.bass as bass
import concourse.mybir as mybir
import concourse.tile as tile
from concourse import bacc
from concourse.bass_utils import run_bass_kernel_spmd

# ---- hardcoded problem geometry ----
B = 8192
DIM1 = 288
DIM2 = 9
CBH = 2592
NCORES = 8
BLOC = B // NCORES          # 1024 batch rows per core
PT = 128                    # partition tile (batch rows per tile)
NT = BLOC // PT             # 8 tiles per core
NCHAN = 32
NROW = 81                   # (g, m1, j) product rows
NQ = 81                     # (g, o3) output rows per channel
CHUNK_C = 4                 # channels per 512-col chunk
NCHUNK = NCHAN // CHUNK_C   # 8 chunks per batch tile
FREE = CHUNK_C * PT         # 512 = matmul moving-dim per chunk

# per group: (col offset in in1, 2*l1+1, gm1 row offset, q offset, D3)
GRP = [(0, 1, 0, 0, 9), (32, 3, 1, 9, 27), (128, 5, 4, 36, 45)]

F32 = mybir.dt.float32
F32R = mybir.dt.float32r
BF16 = mybir.dt.bfloat16
NPBF16 = ml_dtypes.bfloat16

_cache: dict = {}


# --------------------------------------------------------------------------
# Tables from the COO inputs
# --------------------------------------------------------------------------
def _build_tables(cb_vals, i1_idx, i2_idx, out_idx):
    """Build T_all [81, 81], S [9, 81], colmap [81, 32] from the COO triple.

    T_all[p, q]: coefficient taking product row p=(g,m1,j) to output row
    q=(g,o3).  colmap[q, c]: the out column for output row q of channel c.
    Relies on (and verifies) the CG coefficients being channel-independent
    and the out-column order being consistent across channels.
    """
    cb = np.asarray(cb_vals, np.float64)
    i1 = np.asarray(i1_idx, np.int64)
    i2 = np.asarray(i2_idx, np.int64)
    oo = np.asarray(out_idx, np.int64)

    g = np.where(i1 < 32, 0, np.where(i1 < 128, 1, 2))
    rel = i1 - np.array([0, 32, 128])[g]
    width = np.array([1, 3, 5])[g]
    c = rel // width
    m1 = rel % width
    gm1 = np.array([0, 1, 4])[g] + m1
    p = gm1 * 9 + i2

    # distinct out columns per (g, c), sorted ascending -> rank k
    ocols: dict = {}
    for gg, cc, o in zip(g, c, oo):
        ocols.setdefault((int(gg), int(cc)), set()).add(int(o))
    rank: dict = {}
    for (gg, cc), s in ocols.items():
        d3 = GRP[gg][4]
        assert len(s) == d3, f"group {gg} chan {cc}: {len(s)} cols != {d3}"
        for k, o in enumerate(sorted(s)):
            rank[(gg, cc, o)] = k

    T_all = np.zeros((NROW, NQ), np.float64)
    have = np.zeros((NROW, NQ), bool)
    colmap = -np.ones((NQ, NCHAN), np.int64)
    for n in range(len(cb)):
        gg, cc = int(g[n]), int(c[n])
        q = GRP[gg][3] + rank[(gg, cc, int(oo[n]))]
        colmap[q, cc] = oo[n]
        if have[p[n], q]:
            assert abs(T_all[p[n], q] - cb[n]) < 1e-5, "CG not channel-uniform"
        else:
            T_all[p[n], q] = cb[n]
            have[p[n], q] = True
    assert (colmap >= 0).all()
    perm = colmap.reshape(-1)
    assert np.array_equal(np.sort(perm), np.arange(CBH)), "colmap not a perm"

    S = np.zeros((9, NROW), np.float32)
    for pp in range(NROW):
        S[pp // 9, pp] = 1.0
    return T_all.astype(np.float32), S, perm


# --------------------------------------------------------------------------
# Device kernel
# --------------------------------------------------------------------------
def _trace_module():
    nc = bacc.Bacc(trn_type="TRN2")
    in1r = nc.dram_tensor("in1r", [9, NT * NCHAN * PT], BF16, kind="ExternalInput")
    in2r = nc.dram_tensor("in2r", [NROW, BLOC], F32, kind="ExternalInput")
    tmat = nc.dram_tensor("tmat", [NROW, NQ], F32R, kind="ExternalInput")
    smat = nc.dram_tensor("smat", [9, NROW], BF16, kind="ExternalInput")
    outd = nc.dram_tensor("outd", [NQ, NT * NCHAN * PT], BF16, kind="ExternalOutput")

    with tile.TileContext(nc) as tc, ExitStack() as ctx:
        _cg_body(ctx, tc, outd, in1r, in2r, tmat, smat)
    nc.compile()
    return nc


GROUPW = 2                  # chunks per pair-unit
NPAIR = NCHUNK // GROUPW    # 4 pair-units per tile
PSA_BUFS = 2
PSB_BUFS = 2
ZP_BUFS = 4
INP_BUFS = 4
OP_BUFS = 3


def _cg_body(ctx, tc, outd, in1r, in2r, tmat, smat):
    nc = tc.nc
    const = ctx.enter_context(tc.tile_pool(name="const", bufs=1))
    inp = ctx.enter_context(tc.tile_pool(name="inp", bufs=INP_BUFS))
    psa = ctx.enter_context(tc.tile_pool(name="psa", bufs=PSA_BUFS, space="PSUM"))
    psb = ctx.enter_context(tc.tile_pool(name="psb", bufs=PSB_BUFS, space="PSUM"))
    zp = ctx.enter_context(tc.tile_pool(name="zp", bufs=ZP_BUFS))
    op = ctx.enter_context(tc.tile_pool(name="op", bufs=OP_BUFS))

    # gm1-major so matmul rhs slices sit at base partition 0.
    in1v = in1r.ap().rearrange("g (t c b) -> g t c b", t=NT, c=NCHAN)
    sb_in2 = const.tile([NROW, BLOC], F32)
    nc.sync.dma_start(out=sb_in2, in_=in2r.ap())
    sb_t = const.tile([NROW, NQ], F32R)
    nc.sync.dma_start(out=sb_t, in_=tmat.ap())
    sb_s = const.tile([9, NROW], BF16)
    nc.sync.dma_start(out=sb_s, in_=smat.ap())

    lhs_s = sb_s[:]
    lhs_t = sb_t[:]

    for t in range(NT):
        in1t = inp.tile([9, NCHAN, PT], BF16)
        nc.sync.dma_start(out=in1t, in_=in1v[:, t])
        out_sb = op.tile([NQ, NCHUNK, CHUNK_C, PT], BF16)
        last_tile = t == NT - 1
        for pi in range(NPAIR):
            c0 = pi * GROUPW
            # Pair-fused: two 512-col matmuls share one 2-bank PSUM tile so
            # the multiply and evacuation run once per pair at 1024 wide,
            # amortizing their fixed per-op overheads.
            pa = psa.tile([NROW, GROUPW, CHUNK_C, PT], F32)
            for i in range(GROUPW):
                cc = c0 + i
                nc.tensor.matmul(
                    pa[:, i],
                    lhsT=lhs_s,
                    rhs=in1t[:, cc * CHUNK_C:(cc + 1) * CHUNK_C, :],
                    start=True,
                    stop=True,
                )
            z = zp.tile([NROW, GROUPW, CHUNK_C, PT], F32R)
            in2bb = (
                sb_in2[:, t * PT:(t + 1) * PT]
                .unsqueeze(1)
                .unsqueeze(1)
                .broadcast_to((NROW, GROUPW, CHUNK_C, PT))
            )
            nc.vector.tensor_tensor(
                out=z[:], in0=pa[:], in1=in2bb, op=mybir.AluOpType.mult
            )
            pb = psb.tile([NROW, GROUPW, CHUNK_C, PT], F32)
            for i in range(GROUPW):
                nc.tensor.matmul(
                    pb[:, i],
                    lhsT=lhs_t,
                    rhs=z[:, i],
                    start=True,
                    stop=True,
                )
            if last_tile:
                # Fan the final tile's evacuations across engines and store
                # per pair to shorten the pipeline drain.
                evac = [nc.scalar.copy, nc.vector.tensor_copy][pi % 2]
                evac(out=out_sb[:, c0:c0 + GROUPW], in_=pb[:])
                nc.gpsimd.dma_start(
                    out=outd.ap()[:, (t * NCHAN + c0 * CHUNK_C) * PT:
                                  (t * NCHAN + (c0 + GROUPW) * CHUNK_C) * PT],
                    in_=out_sb[:, c0:c0 + GROUPW],
                )
            else:
                nc.scalar.copy(out=out_sb[:, c0:c0 + GROUPW], in_=pb[:])
        if not last_tile:
            # Stores go out on Pool's SWDGE queue so they never queue ahead
            # of the SP-issued in1 loads on the shared DMA dispatch order;
            # half-tile chunks interleave finer on the shared DMA engines.
            half = NCHAN * PT // 2
            nc.gpsimd.dma_start(
                out=outd.ap()[:, t * NCHAN * PT:t * NCHAN * PT + half],
                in_=out_sb[:, :NCHUNK // 2],
            )
            nc.gpsimd.dma_start(
                out=outd.ap()[:, t * NCHAN * PT + half:(t + 1) * NCHAN * PT],
                in_=out_sb[:, NCHUNK // 2:],
            )


def _get_module():
    if "nc" not in _cache:
        _cache["nc"] = _trace_module()
    return _cache["nc"]


# --------------------------------------------------------------------------
# Host glue
# --------------------------------------------------------------------------
def _prep_in1(in1):
    """in1 [B, 288] -> per-core bf16 [9, NT*32*128]: row gm1, free (t, c, b)."""
    g0 = in1[:, 0:32].T[None]                                  # [1, 32, B]
    g1 = in1[:, 32:128].reshape(B, 32, 3).transpose(2, 1, 0)   # [3, 32, B]
    g2 = in1[:, 128:288].reshape(B, 32, 5).transpose(2, 1, 0)  # [5, 32, B]
    r = np.concatenate([g0, g1, g2], axis=0)                   # [9, 32, B]
    cores = []
    for k in range(NCORES):
        rc = r[:, :, k * BLOC:(k + 1) * BLOC].reshape(9, NCHAN, NT, PT)
        rc = rc.transpose(0, 2, 1, 3).reshape(9, NT * NCHAN * PT)
        cores.append(np.ascontiguousarray(rc.astype(NPBF16)))
    return cores


def _prep_in2(in2):
    rep = in2.T[np.arange(NROW) % 9]                           # [81, B]
    return [
        np.ascontiguousarray(rep[:, k * BLOC:(k + 1) * BLOC], np.float32)
        for k in range(NCORES)
    ]


def kernel(in1, in2, cb_vals, i1_idx, i2_idx, out_idx, **run_kwargs):
    in1 = np.asarray(in1, np.float32)
    in2 = np.asarray(in2, np.float32)
    assert in1.shape == (B, DIM1) and in2.shape == (B, DIM2)

    if "tables" not in _cache:
        _cache["tables"] = _build_tables(cb_vals, i1_idx, i2_idx, out_idx)
    t_all, s_mat, perm = _cache["tables"]

    nc = _get_module()
    in1_cores = _prep_in1(in1)
    in2_cores = _prep_in2(in2)
    s_bf = s_mat.astype(NPBF16)
    in_maps = [
        {"in1r": in1_cores[k], "in2r": in2_cores[k], "tmat": t_all, "smat": s_bf}
        for k in range(NCORES)
    ]
    res = run_bass_kernel_spmd(nc, in_maps, core_ids=list(range(NCORES)), **run_kwargs)
    _cache["last_results"] = res

    out = np.empty((B, CBH), np.float32)
    for k in range(NCORES):
        od = np.asarray(res.results[k]["outd"]).astype(np.float32)
        od = od.reshape(NQ, NT, NCHAN, PT)
        oc = od.transpose(1, 3, 0, 2).reshape(BLOC, NQ * NCHAN)
        out[k * BLOC:(k + 1) * BLOC, perm] = oc
    return out
